# revision 1
# baseline (speedup 1.0000x reference)
"""Causal self-attention on 8 TRN2 NeuronCores (Bass/Tile, SPMD).

Problem: B=4, T=2048, C=1024, H=16, D=64, fp32 in/out.

Sharding: core i = (batch b=i//2, parity p=i%2). Each core computes ALL 16
heads for its interleaved quarter of query positions: 256-wide q-chunks
{0,3,4,7} (parity 0) or {1,2,5,6} (parity 1) of batch b. Slot-sorted by
causal prefix length, both parities' slots pad to extents {4,8,12,16}
t'-tiles -> every core runs the IDENTICAL instruction stream (SPMD), with
causality/padding handled by host-supplied mask data. K/V are computed for
the full sequence on both cores of a batch (cheap duplication beats any
collective here). No inter-core communication at all.

Per-core pipeline:
  1. K^T[d,t], Q^T[d,t_own] (d on partitions, heads packed 2/tile) and
     V_aug[t,(h,d|1)] (ones column folded in for softmax sums) via fp32r
     matmuls (1 cyc/row at N>=256; measured rel err 1.5e-4 at K=1024).
  2. Flash-style attention per (head-pair, q-slot): S^T = K @ Q^T with
     2-head row-packed matmuls (K=64 via tile_position), batched exp on
     ScalarE over [128,1024] PSUM spans, causal mask-mul on DVE for the
     last 4 t'-tiles of each slot, PV accumulation O^T = V_aug.T @ P^T
     (M=65: row 64 = softmax denominators l). Normalize with fast DVE
     reciprocal + GpSimd partition-broadcast.
  3. Output projection Y = O^T.T @ Wproj + bias_eff, where
     bias_eff = bproj + bv @ Wproj (V-bias folded in exactly since
     softmax rows sum to 1).

Host: slices/transposes inputs, precomputes masks + effective bias,
reassembles the 8 per-core [1024,1024] outputs into [4,2048,1024].
"""
import os
import numpy as np

import concourse.bacc as bacc
import concourse.mybir as mybir
import concourse.tile as tile
from concourse.bass_utils import run_bass_kernel_spmd

B, T, C, H, D = 4, 2048, 1024, 16, 64
QC = 256                      # q-chunk width
NSLOT = 4                     # q-chunks per core
OWN = [[0, 3, 4, 7], [1, 2, 5, 6]]   # global q-chunk ids per parity, slot order
EXT = [4, 8, 12, 16]          # padded t'-tile (128) extent per slot
F32 = mybir.dt.float32
F32R = mybir.dt.float32r
VA_W = H * (D + 1)            # 1040: V_aug cols = 16 heads x (64 + ones)

_cache = {}


def _build():
    nc = bacc.Bacc("TRN2", target_bir_lowering=False, debug=False,
                   enable_asserts=False, num_devices=8)
    dt_in = {}
    def din(name, shape):
        dt_in[name] = nc.dram_tensor(name, list(shape), F32, kind="ExternalInput").ap()
        return dt_in[name]

    xt_d = din("xt", (C, T))            # x[b].T
    xq_d = din("xq", (C, NSLOT * QC))   # own q columns of xt
    wq_d = din("wq", (C, C))            # pre-scaled by 1/8
    wk_d = din("wk", (C, C))
    wv_d = din("wv", (C, C))
    wp_d = din("wp", (C, C))
    bq_d = din("bq", (8, 128, 1))
    bk_d = din("bk", (8, 128, 1))
    bpeb_d = din("bpeb", (128, C))      # bproj_eff broadcast to 128 partitions
    mk_d = din("masks", (NSLOT, 4, 128, QC))
    y_d = nc.dram_tensor("y", [NSLOT * QC, C], F32, kind="ExternalOutput").ap()
    qt_d = nc.dram_tensor("qt", [C, NSLOT * QC], F32R).ap()
    ot_d = nc.dram_tensor("ot", [C, NSLOT * QC], F32R).ap()

    bypass = mybir.AluOpType.bypass
    mult = mybir.AluOpType.mult
    add = mybir.AluOpType.add
    EXP = mybir.ActivationFunctionType.Exp

    with tile.TileContext(nc) as tc:
        # ---------------- persistent K^T tiles --------------------------
        ktp = tc.alloc_tile_pool(name="ktp", bufs=1)
        KT = [ktp.tile([128, T], F32R, name=f"kt{j}", tag=f"kt{j}") for j in range(8)]

        # ---------------- phase 1a: K^T and Q^T -------------------------
        with tc.tile_pool(name="p1a", bufs=1) as wpool, \
             tc.tile_pool(name="p1ax", bufs=2) as xsp, \
             tc.tile_pool(name="p1ae", bufs=3) as evp, \
             tc.tile_pool(name="p1ap", bufs=1, space="PSUM") as ps1:
            wkc = [wpool.tile([128, C], F32R, name=f"wkc{c}", tag=f"wkc{c}") for c in range(8)]
            wqc = [wpool.tile([128, C], F32R, name=f"wqc{c}", tag=f"wqc{c}") for c in range(8)]
            bks = [wpool.tile([128, 1], F32, name=f"bks{j}", tag=f"bks{j}") for j in range(8)]
            bqs = [wpool.tile([128, 1], F32, name=f"bqs{j}", tag=f"bqs{j}") for j in range(8)]
            for c in range(8):
                nc.sync.dma_start(out=wkc[c][:], in_=wk_d[128*c:128*(c+1), :].bitcast(F32R))
                nc.sync.dma_start(out=wqc[c][:], in_=wq_d[128*c:128*(c+1), :].bitcast(F32R))
                nc.sync.dma_start(out=bks[c][:], in_=bk_d[c])
                nc.sync.dma_start(out=bqs[c][:], in_=bq_d[c])
            # K^T: 4 t-slabs of 512
            for slab in range(4):
                xts = []
                for c in range(8):
                    xt_t = xsp.tile([128, 512], F32R, name=f"xts{c}", tag=f"xts{c}")
                    nc.sync.dma_start(out=xt_t[:], in_=xt_d[128*c:128*(c+1), 512*slab:512*(slab+1)].bitcast(F32R))
                    xts.append(xt_t)
                pks = [ps1.tile([128, 512], F32, name=f"pk{j}", tag=f"pk{j}") for j in range(8)]
                for c in range(8):
                    for j in range(8):
                        nc.tensor.matmul(out=pks[j][:], lhsT=wkc[c][:, 128*j:128*(j+1)],
                                         rhs=xts[c][:], start=(c == 0), stop=(c == 7))
                for j in range(8):
                    nc.vector.tensor_scalar_add(out=KT[j][:, 512*slab:512*(slab+1)],
                                                in0=pks[j][:], scalar1=bks[j][:])
            # Q^T: 2 t-slabs of 512 over own columns
            for slab in range(2):
                xqs = []
                for c in range(8):
                    xq_t = xsp.tile([128, 512], F32R, name=f"xts{c}", tag=f"xts{c}")
                    nc.sync.dma_start(out=xq_t[:], in_=xq_d[128*c:128*(c+1), 512*slab:512*(slab+1)].bitcast(F32R))
                    xqs.append(xq_t)
                pqs = [ps1.tile([128, 512], F32, name=f"pk{j}", tag=f"pk{j}") for j in range(8)]
                for c in range(8):
                    for j in range(8):
                        nc.tensor.matmul(out=pqs[j][:], lhsT=wqc[c][:, 128*j:128*(j+1)],
                                         rhs=xqs[c][:], start=(c == 0), stop=(c == 7))
                for j in range(8):
                    qsb = evp.tile([128, 512], F32R, name="qsb", tag="qsb")
                    nc.vector.tensor_scalar_add(out=qsb[:], in0=pqs[j][:], scalar1=bqs[j][:])
                    nc.sync.dma_start(out=qt_d[128*j:128*(j+1), 512*slab:512*(slab+1)], in_=qsb[:])

        # ---------------- phase 1b: V_aug -------------------------------
        vap = tc.alloc_tile_pool(name="vap", bufs=1)
        VA = [vap.tile([128, VA_W], F32R, name=f"va{g}", tag=f"va{g}") for g in range(16)]
        with tc.tile_pool(name="p1b", bufs=1) as wvp, \
             tc.tile_pool(name="p1bx", bufs=1) as xsp2, \
             tc.tile_pool(name="p1bp", bufs=1, space="PSUM") as ps2:
            wvc = [wvp.tile([128, C], F32R, name=f"wvc{c}", tag=f"wvc{c}") for c in range(8)]
            for c in range(8):
                nc.sync.dma_start(out=wvc[c][:], in_=wv_d[128*c:128*(c+1), :].bitcast(F32R))
            ones16 = wvp.tile([128, H], F32, name="ones16", tag="ones16")
            nc.vector.memset(ones16[:], 1.0)
            ones16_3d = ones16[:].unsqueeze(2)
            for g in range(16):
                dst1 = VA[g][:].rearrange("p (h d) -> p h d", d=D+1)[:, :, D:D+1]
                nc.vector.tensor_copy(out=dst1, in_=ones16_3d)
            for slab in range(4):
                xts2 = []
                for c in range(8):
                    xv_t = xsp2.tile([128, 512], F32R, name=f"xv{c}", tag=f"xv{c}")
                    nc.sync.dma_start(out=xv_t[:], in_=xt_d[128*c:128*(c+1), 512*slab:512*(slab+1)].bitcast(F32R))
                    xts2.append(xv_t)
                pvs = [ps2.tile([128, 512], F32, name=f"pv{u}", tag=f"pv{u}") for u in range(8)]
                for c in range(8):
                    for tt in range(4):
                        for jc in range(2):
                            nc.tensor.matmul(out=pvs[tt*2+jc][:],
                                             lhsT=xts2[c][:, 128*tt:128*(tt+1)],
                                             rhs=wvc[c][:, 512*jc:512*(jc+1)],
                                             start=(c == 0), stop=(c == 7))
                for tt in range(4):
                    g = 4*slab + tt
                    for jc in range(2):
                        dst = VA[g][:, 520*jc:520*(jc+1)].rearrange("p (h d) -> p h d", d=D+1)[:, :, 0:D]
                        src = pvs[tt*2+jc][:].rearrange("p (h d) -> p h d", d=D)
                        nc.vector.tensor_copy(out=dst, in_=src)

        # ---------------- phase 2: attention ----------------------------
        with tc.tile_pool(name="mkp", bufs=1) as mkp, \
             tc.tile_pool(name="qrp", bufs=3) as qrp, \
             tc.tile_pool(name="ptp", bufs=3) as ptp, \
             tc.tile_pool(name="smp", bufs=2) as smp, \
             tc.tile_pool(name="p2p", bufs=1, space="PSUM") as psa:
            MK = []
            for s in range(NSLOT):
                row = []
                for mi in range(4):
                    mt = mkp.tile([128, QC], F32R, name=f"mk{s}{mi}", tag=f"mk{s}{mi}")
                    nc.sync.dma_start(out=mt[:], in_=mk_d[s, mi].bitcast(F32R))
                    row.append(mt)
                MK.append(row)
            for s in range(NSLOT):
                E = EXT[s]
                for j in range(8):
                    qr = qrp.tile([128, QC], F32R, name="qr", tag="qr")
                    nc.sync.dma_start(out=qr[:], in_=qt_d[128*j:128*(j+1), QC*s:QC*(s+1)])
                    oa = psa.tile([65, QC], F32, name="oa", tag="oa", bufs=2)
                    ob = psa.tile([65, QC], F32, name="ob", tag="ob", bufs=2)
                    for g in range(E // 2):
                        ss = psa.tile([128, 4*QC], F32, name="ss", tag="ss", bufs=2)
                        for u in range(2):
                            m = 2*g + u
                            nc.tensor.matmul(out=ss[:, QC*u:QC*(u+1)],
                                             lhsT=KT[j][0:64, 128*m:128*(m+1)],
                                             rhs=qr[0:64, :], tile_position=(0, 0),
                                             start=True, stop=True)
                            nc.tensor.matmul(out=ss[:, 2*QC+QC*u:2*QC+QC*(u+1)],
                                             lhsT=KT[j][64:128, 128*m:128*(m+1)],
                                             rhs=qr[64:128, :], tile_position=(64, 0),
                                             start=True, stop=True)
                        pt = ptp.tile([128, 4*QC], F32R, name="pt", tag="pt")
                        nc.scalar.activation(out=pt[:], in_=ss[:], func=EXP)
                        for u in range(2):
                            m = 2*g + u
                            for half, h in ((0, 2*j), (1, 2*j + 1)):
                                pcol = (2*half + u) * QC
                                psl = pt[:, pcol:pcol+QC]
                                if m >= E - 4:
                                    nc.vector.scalar_tensor_tensor(
                                        out=psl, in0=psl, scalar=0.0, in1=MK[s][m-(E-4)][:],
                                        op0=bypass, op1=mult)
                                nc.tensor.matmul(out=(oa if half == 0 else ob)[:],
                                                 lhsT=VA[m][:, 65*h:65*(h+1)],
                                                 rhs=psl,
                                                 start=(m == 0), stop=(m == E - 1))
                    # normalize: r = 1/l, broadcast, multiply; write O^T
                    for half, (acc, h) in enumerate(((oa, 2*j), (ob, 2*j + 1))):
                        lsb = smp.tile([1, QC], F32, name="lsb", tag=f"lsb{half}")
                        nc.vector.tensor_copy(out=lsb[:], in_=acc[64:65, :])
                        rsb = smp.tile([1, QC], F32, name="rsb", tag=f"rsb{half}")
                        nc.vector.reciprocal_approx_fast(rsb[:], lsb[:])
                        rbb = smp.tile([64, QC], F32, name="rbb", tag=f"rbb{half}")
                        nc.gpsimd.partition_broadcast(rbb[:], rsb[:])
                        osb = smp.tile([64, QC], F32R, name="osb", tag=f"osb{half}")
                        nc.vector.scalar_tensor_tensor(out=osb[:], in0=acc[0:64, :],
                                                       scalar=0.0, in1=rbb[:],
                                                       op0=bypass, op1=mult)
                        nc.sync.dma_start(out=ot_d[64*h:64*(h+1), QC*s:QC*(s+1)], in_=osb[:])

        # ---------------- phase 3: output projection --------------------
        with tc.tile_pool(name="p3w", bufs=1) as wpp, \
             tc.tile_pool(name="p3o", bufs=2) as lop, \
             tc.tile_pool(name="p3y", bufs=3) as yp, \
             tc.tile_pool(name="p3p", bufs=2, space="PSUM") as ps3:
            wpc = [wpp.tile([128, C], F32R, name=f"wpc{c}", tag=f"wpc{c}") for c in range(8)]
            bpeb = wpp.tile([128, C], F32, name="bpeb", tag="bpeb")
            nc.sync.dma_start(out=bpeb[:], in_=bpeb_d[:])
            for c in range(8):
                nc.sync.dma_start(out=wpc[c][:], in_=wp_d[128*c:128*(c+1), :].bitcast(F32R))
            for ti in range(8):
                lots = []
                for c in range(8):
                    lot = lop.tile([128, 128], F32R, name=f"lot{c}", tag=f"lot{c}")
                    nc.sync.dma_start(out=lot[:], in_=ot_d[128*c:128*(c+1), 128*ti:128*(ti+1)])
                    lots.append(lot)
                for jc in range(2):
                    py = ps3.tile([128, 512], F32, name="py", tag="py")
                    for c in range(8):
                        nc.tensor.matmul(out=py[:], lhsT=lots[c][:],
                                         rhs=wpc[c][:, 512*jc:512*(jc+1)],
                                         start=(c == 0), stop=(c == 7))
                    ysb = yp.tile([128, 512], F32, name="ysb", tag="ysb")
                    nc.vector.scalar_tensor_tensor(out=ysb[:], in0=py[:], scalar=0.0,
                                                   in1=bpeb[:, 512*jc:512*(jc+1)],
                                                   op0=bypass, op1=add)
                    nc.sync.dma_start(out=y_d[128*ti:128*(ti+1), 512*jc:512*(jc+1)], in_=ysb[:])
        vap.release()
        ktp.release()

    nc.compile()
    return nc


def _get_nc():
    if "nc" not in _cache:
        _cache["nc"] = _build()
    return _cache["nc"]


def _host_prep(x, Wqkv, bqkv, Wproj, bproj):
    x = np.ascontiguousarray(np.asarray(x, dtype=np.float32))
    Wqkv = np.asarray(Wqkv, dtype=np.float32)
    bqkv = np.asarray(bqkv, dtype=np.float32)
    Wproj = np.ascontiguousarray(np.asarray(Wproj, dtype=np.float32))
    bproj = np.asarray(bproj, dtype=np.float32)

    wq = np.ascontiguousarray(Wqkv[:, :C] * np.float32(0.125))
    wk = np.ascontiguousarray(Wqkv[:, C:2*C])
    wv = np.ascontiguousarray(Wqkv[:, 2*C:])
    bq8 = (bqkv[:C] * np.float32(0.125)).reshape(8, 128, 1).copy()
    bk8 = bqkv[C:2*C].reshape(8, 128, 1).copy()
    bv = bqkv[2*C:]
    bpe = (bproj.astype(np.float64) + bv.astype(np.float64) @ Wproj.astype(np.float64)).astype(np.float32)
    bpeb = np.ascontiguousarray(np.broadcast_to(bpe, (128, C)))

    pidx = np.arange(128)[:, None]
    fidx = np.arange(QC)[None, :]
    masks = []
    for par in range(2):
        mk = np.zeros((NSLOT, 4, 128, QC), dtype=np.float32)
        for s, cchunk in enumerate(OWN[par]):
            for mi in range(4):
                g = EXT[s] - 4 + mi
                mk[s, mi] = ((128*g + pidx) <= (QC*cchunk + fidx)).astype(np.float32)
        masks.append(mk)

    in_maps = []
    for core in range(8):
        b, par = core // 2, core % 2
        xt = np.ascontiguousarray(x[b].T)
        xq = np.ascontiguousarray(
            np.concatenate([xt[:, QC*c:QC*(c+1)] for c in OWN[par]], axis=1))
        in_maps.append(dict(xt=xt, xq=xq, wq=wq, wk=wk, wv=wv, wp=Wproj,
                            bq=bq8, bk=bk8, bpeb=bpeb, masks=masks[par]))
    return in_maps


def kernel(x, Wqkv, bqkv, Wproj, bproj):
    nc = _get_nc()
    in_maps = _host_prep(x, Wqkv, bqkv, Wproj, bproj)
    trace = bool(os.environ.get("BASS_TRACE"))
    res = run_bass_kernel_spmd(nc, in_maps, list(range(8)), trace=trace)
    _cache["last_exec_time_ns"] = res.exec_time_ns
    _cache["last_res"] = res
    out = np.empty((B, T, C), dtype=np.float32)
    for core in range(8):
        b, par = core // 2, core % 2
        y = res.results[core]["y"]
        for s, cchunk in enumerate(OWN[par]):
            out[b, QC*cchunk:QC*(cchunk+1)] = y[QC*s:QC*(s+1)]
    return out



# revision 5
# speedup vs baseline: 1.6011x; 1.6011x over previous
"""Causal self-attention on 8 TRN2 NeuronCores (Bass/Tile, SPMD).

Problem: B=4, T=2048, C=1024, H=16, D=64, fp32 in/out.

Sharding: core i = (batch b=i//2, parity p=i%2). Each core computes ALL 16
heads for its interleaved quarter of query positions: 256-wide q-chunks
{0,3,4,7} (parity 0) or {1,2,5,6} (parity 1) of batch b, slot-sorted by
causal prefix so both parities' slots pad to extents {4,8,12,16} key-tiles
of 128 -> every core runs the IDENTICAL instruction stream (SPMD); the
causal mask is host-supplied data. No inter-core communication.

v2 vs baseline (668us):
 - bf16 for all matmul operands (sim rel-err 0.53% vs 2e-2 gate); halves
   DMA + SBUF, keeps Q^T and O^T resident (no DRAM roundtrips).
 - Causal mask folded into PSUM *before* exp as an additive (0/-30)
   identity-matmul accumulate on TensorE (start of the S accumulation
   group) instead of ~256 DVE multiplies after exp.
 - One x^T pass feeds both K^T and V projections.
 - K/V projection slab s is emitted right before attention slot s
   (EXT[s] = 4(s+1) key-tiles = exactly slabs 0..s), so ScalarE exp
   overlaps projection matmuls and the PE never idles long enough to
   re-throttle (HAM).
 - Output projection reads O^T straight from SBUF.
"""
import os
import numpy as np
import ml_dtypes

import concourse.bacc as bacc
import concourse.mybir as mybir
import concourse.tile as tile
from concourse.bass_utils import run_bass_kernel_spmd

B, T, C, H, D = 4, 2048, 1024, 16, 64
QC = 256                      # q-chunk width
NSLOT = 4                     # q-chunks per core
OWN = [[0, 3, 4, 7], [1, 2, 5, 6]]   # global q-chunk ids per parity, slot order
EXT = [4, 8, 12, 16]          # padded key-tile (128) extent per slot
F32 = mybir.dt.float32
BF16 = mybir.dt.bfloat16
VA_W = H * (D + 1)            # 1040: V_aug cols = 16 heads x (64 | ones)
NEG = -30.0                   # additive mask for causally-forbidden keys

_cache = {}


def _build():
    nc = bacc.Bacc("TRN2", target_bir_lowering=False, debug=False,
                   enable_asserts=False, num_devices=8)

    def din(name, shape, dt=BF16):
        return nc.dram_tensor(name, list(shape), dt, kind="ExternalInput").ap()

    xt_d = din("xt", (C, T))            # x[b].T
    xq_d = din("xq", (C, NSLOT * QC))   # own q columns of x[b].T
    wq_d = din("wq", (C, C))            # pre-scaled by 1/8
    wk_d = din("wk", (C, C))
    wv_d = din("wv", (C, C))
    wp_d = din("wp", (C, C))
    bq_d = din("bq", (8, 128, 1), F32)  # pre-scaled by 1/8
    bk_d = din("bk", (8, 128, 1), F32)
    bpeb_d = din("bpeb", (128, C), F32)  # bproj_eff broadcast to 128 partitions
    mk_d = din("masks", (NSLOT, 4, 128, QC))  # additive 0/-30
    id_d = din("id128", (128, 128))
    y_d = nc.dram_tensor("y", [NSLOT * QC, C], F32, kind="ExternalOutput").ap()

    bypass = mybir.AluOpType.bypass
    mult = mybir.AluOpType.mult
    add = mybir.AluOpType.add
    EXP = mybir.ActivationFunctionType.Exp

    with tile.TileContext(nc) as tc:
        # ---------------- persistent tiles ------------------------------
        pers = tc.alloc_tile_pool(name="pers", bufs=1)
        # K^T per (j, slab): [d of heads 2j,2j+1; 512 keys]
        KT = [[pers.tile([128, 512], BF16, name=f"kt{j}_{sl}", tag=f"kt{j}_{sl}")
               for sl in range(4)] for j in range(8)]
        # Q^T per (j, slab of own q): [d; 512 own-q]
        QT = [[pers.tile([128, 512], BF16, name=f"qt{j}_{sl}", tag=f"qt{j}_{sl}")
               for sl in range(2)] for j in range(8)]
        # V_aug per key-tile g: [128 keys; 16 heads x (64 d | ones)]
        VA = [pers.tile([128, VA_W], BF16, name=f"va{g}", tag=f"va{g}")
              for g in range(16)]
        # O^T per (j, slot): rows 0-63 head 2j, 64-127 head 2j+1
        OT = [[pers.tile([128, QC], BF16, name=f"ot{j}_{s}", tag=f"ot{j}_{s}")
               for s in range(NSLOT)] for j in range(8)]
        MK = [[pers.tile([128, QC], BF16, name=f"mk{s}{mi}", tag=f"mk{s}{mi}")
               for mi in range(4)] for s in range(NSLOT)]
        ID = pers.tile([128, 128], BF16, name="id128", tag="id128")
        nc.sync.dma_start(out=ID[:], in_=id_d)
        for s in range(NSLOT):
            for mi in range(4):
                nc.sync.dma_start(out=MK[s][mi][:], in_=mk_d[s, mi])
        ones16 = pers.tile([128, H], BF16, name="ones16", tag="ones16")
        nc.vector.memset(ones16[:], 1.0)
        ones16_3d = ones16[:].unsqueeze(2)
        for g in range(16):
            dst1 = VA[g][:].rearrange("p (h d) -> p h d", d=D + 1)[:, :, D:D + 1]
            nc.vector.tensor_copy(out=dst1, in_=ones16_3d)

        # ---------------- Q phase ---------------------------------------
        with tc.tile_pool(name="qw", bufs=1) as qw, \
             tc.tile_pool(name="qx", bufs=2) as qx, \
             tc.tile_pool(name="qps", bufs=1, space="PSUM") as qps:
            wqt = [qw.tile([128, C], BF16, name=f"wq{c}", tag=f"wq{c}") for c in range(8)]
            bqs = [qw.tile([128, 1], F32, name=f"bq{j}", tag=f"bq{j}") for j in range(8)]
            for c in range(8):
                nc.sync.dma_start(out=wqt[c][:], in_=wq_d[128*c:128*(c+1), :])
                nc.sync.dma_start(out=bqs[c][:], in_=bq_d[c])
            for slab in range(2):
                xqs = []
                for c in range(8):
                    t = qx.tile([128, 512], BF16, name=f"xq{c}", tag=f"xq{c}")
                    nc.sync.dma_start(out=t[:], in_=xq_d[128*c:128*(c+1), 512*slab:512*(slab+1)])
                    xqs.append(t)
                for w in range(4):
                    pq = [qps.tile([128, 512], F32, name=f"q{i}", tag=f"q{i}") for i in range(2)]
                    for c in range(8):
                        for i in range(2):
                            nc.tensor.matmul(out=pq[i][:], lhsT=wqt[c][:, 128*(2*w+i):128*(2*w+i+1)],
                                             rhs=xqs[c][:], start=(c == 0), stop=(c == 7))
                    for i in range(2):
                        j = 2 * w + i
                        nc.vector.tensor_scalar_add(out=QT[j][slab][:], in0=pq[i][:],
                                                    scalar1=bqs[j][:])

        # ---------------- K/V slabs interleaved with attention ----------
        with tc.tile_pool(name="kvw", bufs=1) as kvw, \
             tc.tile_pool(name="xtp", bufs=2) as xtp, \
             tc.tile_pool(name="ptp", bufs=3) as ptp, \
             tc.tile_pool(name="smp", bufs=2) as smp, \
             tc.tile_pool(name="kvps", bufs=1, space="PSUM") as kvps, \
             tc.tile_pool(name="aps", bufs=1, space="PSUM") as aps:
            wkt = [kvw.tile([128, C], BF16, name=f"wk{c}", tag=f"wk{c}") for c in range(8)]
            wvt = [kvw.tile([128, C], BF16, name=f"wv{c}", tag=f"wv{c}") for c in range(8)]
            bks = [kvw.tile([128, 1], F32, name=f"bk{j}", tag=f"bk{j}") for j in range(8)]
            for c in range(8):
                nc.sync.dma_start(out=wkt[c][:], in_=wk_d[128*c:128*(c+1), :])
                nc.sync.dma_start(out=wvt[c][:], in_=wv_d[128*c:128*(c+1), :])
                nc.sync.dma_start(out=bks[c][:], in_=bk_d[c])

            for s in range(NSLOT):
                slab = s
                E = EXT[s]
                # ---- K^T + V for key slab s (keys 512s..512s+512) ----
                xts = []
                for c in range(8):
                    t = xtp.tile([128, 512], BF16, name=f"xt{c}", tag=f"xt{c}")
                    nc.sync.dma_start(out=t[:], in_=xt_d[128*c:128*(c+1), 512*slab:512*(slab+1)])
                    xts.append(t)
                for w in range(4):      # K waves: j pairs
                    pk = [kvps.tile([128, 512], F32, name=f"kv{i}", tag=f"kv{i}") for i in range(2)]
                    for c in range(8):
                        for i in range(2):
                            j = 2 * w + i
                            nc.tensor.matmul(out=pk[i][:], lhsT=wkt[c][:, 128*j:128*(j+1)],
                                             rhs=xts[c][:], start=(c == 0), stop=(c == 7))
                    for i in range(2):
                        j = 2 * w + i
                        nc.vector.tensor_scalar_add(out=KT[j][slab][:], in0=pk[i][:],
                                                    scalar1=bks[j][:])
                for tt in range(4):     # V waves: jc pairs per t-subtile
                    g = 4 * slab + tt
                    pv = [kvps.tile([128, 512], F32, name=f"kv{i}", tag=f"kv{i}") for i in range(2)]
                    for c in range(8):
                        for jc in range(2):
                            nc.tensor.matmul(out=pv[jc][:],
                                             lhsT=xts[c][:, 128*tt:128*(tt+1)],
                                             rhs=wvt[c][:, 512*jc:512*(jc+1)],
                                             start=(c == 0), stop=(c == 7))
                    for jc in range(2):
                        dst = VA[g][:, 520*jc:520*(jc+1)].rearrange("p (h d) -> p h d", d=D+1)[:, :, 0:D]
                        src = pv[jc][:].rearrange("p (h d) -> p h d", d=D)
                        nc.vector.tensor_copy(out=dst, in_=src)

                # ---- attention slot s (uses key tiles 0..E-1) ----
                for j in range(8):
                    o2 = aps.tile([65, 512], F32, name="o2", tag="o2", bufs=2)
                    for g in range(E // 2):
                        masked = (2 * g) >= E - 4
                        ss = aps.tile([128, 1024], F32, name="ss", tag="ss", bufs=2)
                        for u in range(2):
                            m = 2 * g + u
                            sl, mm = m // 4, m % 4
                            if masked:
                                mi = m - (E - 4)
                                for h in range(2):
                                    nc.tensor.matmul(out=ss[:, 512*h+QC*u:512*h+QC*(u+1)],
                                                     lhsT=ID[:], rhs=MK[s][mi][:],
                                                     start=True, stop=False,
                                                     skip_group_check=True)
                            for h in range(2):
                                nc.tensor.matmul(
                                    out=ss[:, 512*h+QC*u:512*h+QC*(u+1)],
                                    lhsT=KT[j][sl][64*h:64*(h+1), 128*mm:128*(mm+1)],
                                    rhs=QT[j][s // 2][64*h:64*(h+1), QC*(s % 2):QC*(s % 2 + 1)],
                                    tile_position=(64 * h, 0),
                                    start=(not masked), stop=True,
                                    skip_group_check=masked)
                        pt = ptp.tile([128, 1024], BF16, name="pt", tag="pt")
                        nc.scalar.activation(out=pt[:], in_=ss[:], func=EXP)
                        for u in range(2):
                            m = 2 * g + u
                            for h in range(2):
                                nc.tensor.matmul(out=o2[:, QC*h:QC*(h+1)],
                                                 lhsT=VA[m][:, 65*(2*j+h):65*(2*j+h)+65],
                                                 rhs=pt[:, 512*h+QC*u:512*h+QC*(u+1)],
                                                 start=(m == 0 and h == 0),
                                                 stop=(m == E - 1),
                                                 skip_group_check=True)
                    # normalize: r = 1/l broadcast down partitions, scale O^T
                    lsb = smp.tile([1, 512], F32, name="lsb", tag="lsb")
                    nc.vector.tensor_copy(out=lsb[:], in_=o2[64:65, :])
                    rsb = smp.tile([1, 512], F32, name="rsb", tag="rsb")
                    nc.vector.reciprocal_approx_fast(rsb[:], lsb[:])
                    rbb = smp.tile([64, 512], F32, name="rbb", tag="rbb")
                    nc.gpsimd.partition_broadcast(rbb[:], rsb[:])
                    for h in range(2):
                        nc.vector.scalar_tensor_tensor(
                            out=OT[j][s][64*h:64*(h+1), :], in0=o2[0:64, QC*h:QC*(h+1)],
                            scalar=0.0, in1=rbb[:, QC*h:QC*(h+1)],
                            op0=bypass, op1=mult)

        # ---------------- output projection -----------------------------
        with tc.tile_pool(name="pw", bufs=1) as pw, \
             tc.tile_pool(name="yp", bufs=3) as yp, \
             tc.tile_pool(name="pps", bufs=2, space="PSUM") as pps:
            wpt = [pw.tile([128, C], BF16, name=f"wp{c}", tag=f"wp{c}") for c in range(8)]
            bpeb = pw.tile([128, C], F32, name="bpeb", tag="bpeb")
            nc.sync.dma_start(out=bpeb[:], in_=bpeb_d[:])
            for c in range(8):
                nc.sync.dma_start(out=wpt[c][:], in_=wp_d[128*c:128*(c+1), :])
            for ti in range(8):
                s, half = ti // 2, ti % 2
                for jc in range(2):
                    py = pps.tile([128, 512], F32, name="py", tag="py")
                    for c in range(8):
                        nc.tensor.matmul(out=py[:],
                                         lhsT=OT[c][s][:, 128*half:128*(half+1)],
                                         rhs=wpt[c][:, 512*jc:512*(jc+1)],
                                         start=(c == 0), stop=(c == 7))
                    ysb = yp.tile([128, 512], F32, name="ysb", tag="ysb")
                    nc.vector.scalar_tensor_tensor(out=ysb[:], in0=py[:], scalar=0.0,
                                                   in1=bpeb[:, 512*jc:512*(jc+1)],
                                                   op0=bypass, op1=add)
                    nc.sync.dma_start(out=y_d[128*ti:128*(ti+1), 512*jc:512*(jc+1)], in_=ysb[:])
        pers.release()

    nc.compile()
    return nc


def _get_nc():
    if "nc" not in _cache:
        _cache["nc"] = _build()
    return _cache["nc"]


def _host_prep(x, Wqkv, bqkv, Wproj, bproj):
    bf = ml_dtypes.bfloat16
    x = np.ascontiguousarray(np.asarray(x, dtype=np.float32))
    Wqkv = np.asarray(Wqkv, dtype=np.float32)
    bqkv = np.asarray(bqkv, dtype=np.float32)
    Wproj = np.ascontiguousarray(np.asarray(Wproj, dtype=np.float32))
    bproj = np.asarray(bproj, dtype=np.float32)

    wq = np.ascontiguousarray(Wqkv[:, :C] * np.float32(0.125)).astype(bf)
    wk = np.ascontiguousarray(Wqkv[:, C:2*C]).astype(bf)
    wv = np.ascontiguousarray(Wqkv[:, 2*C:]).astype(bf)
    wp = Wproj.astype(bf)
    bq8 = (bqkv[:C] * np.float32(0.125)).reshape(8, 128, 1).copy()
    bk8 = bqkv[C:2*C].reshape(8, 128, 1).copy()
    bv = bqkv[2*C:]
    bpe = (bproj.astype(np.float64) + bv.astype(np.float64) @ Wproj.astype(np.float64)).astype(np.float32)
    bpeb = np.ascontiguousarray(np.broadcast_to(bpe, (128, C)))
    id128 = np.eye(128, dtype=bf)

    pidx = np.arange(128)[:, None]
    fidx = np.arange(QC)[None, :]
    masks = []
    for par in range(2):
        mk = np.zeros((NSLOT, 4, 128, QC), dtype=np.float32)
        for s, cchunk in enumerate(OWN[par]):
            for mi in range(4):
                g = EXT[s] - 4 + mi
                mk[s, mi] = np.where((128*g + pidx) <= (QC*cchunk + fidx), 0.0, NEG)
        masks.append(mk.astype(bf))

    in_maps = []
    for core in range(8):
        b, par = core // 2, core % 2
        xt = np.ascontiguousarray(x[b].T)
        xq = np.ascontiguousarray(
            np.concatenate([xt[:, QC*c:QC*(c+1)] for c in OWN[par]], axis=1)).astype(bf)
        in_maps.append(dict(xt=xt.astype(bf), xq=xq, wq=wq, wk=wk, wv=wv, wp=wp,
                            bq=bq8, bk=bk8, bpeb=bpeb, masks=masks[par],
                            id128=id128))
    return in_maps


def kernel(x, Wqkv, bqkv, Wproj, bproj):
    nc = _get_nc()
    in_maps = _host_prep(x, Wqkv, bqkv, Wproj, bproj)
    trace = bool(os.environ.get("BASS_TRACE"))
    res = run_bass_kernel_spmd(nc, in_maps, list(range(8)), trace=trace)
    _cache["last_exec_time_ns"] = res.exec_time_ns
    _cache["last_res"] = res
    out = np.empty((B, T, C), dtype=np.float32)
    for core in range(8):
        b, par = core // 2, core % 2
        y = res.results[core]["y"]
        for s, cchunk in enumerate(OWN[par]):
            out[b, QC*cchunk:QC*(cchunk+1)] = y[QC*s:QC*(s+1)]
    return out
